# revision 4
# baseline (speedup 1.0000x reference)
"""Trainium2 Bass kernel for 2-layer ARMA GCN (nn_Net_33586644255234).

Strategy (graph/data parallel over 8 NeuronCores):
  - Nodes padded 40000 -> 40960 and sharded 5120/core (40 tiles of 128).
  - Weights replicated; per-core x^T shard shipped pre-transposed bf16.
  - Per layer:
      h' = dinv * (x @ W_init)          (dense, PE; dinv scale fused on ACT)
      AllGather h' -> full node table in every core's HBM
      per dst-tile: gather edge messages h'[src] with gpsimd.dma_gather,
      scatter-add via one-hot matmul:  psum += S_chunk^T @ msgs_chunk
      out = relu(dinv * psum + x @ W_root + b)
  - Edge bookkeeping (sort by dst tile, split by src half for int16 gather
    indices, chunk grids uniform across cores) is host-side sharding prep.

kernel(**inputs) takes FULL inputs, returns FULL [40000, 64] float32.
"""

import os
import sys

sys.path.insert(0, "/opt/trn_rl_repo")

import numpy as np
import ml_dtypes

import concourse.bass as bass
import concourse.mybir as mybir
import concourse.tile as tile
from concourse import bacc
from concourse.bass_utils import run_bass_kernel_spmd

# ---------------- problem constants (hardcoded per contract) ----------------
N, E, F_IN, HID, NCLS = 40000, 640000, 512, 128, 64
P = 128
NCORES = 8
NPC = 5120          # nodes per core (padded)
NPAD = NCORES * NPC  # 40960
TPC = NPC // P       # 40 dst tiles per core
KT = F_IN // P       # 4 k-tiles for layer-1 dense
HALF = NPAD // 2     # 20480 (int16-safe gather table half)
SB = 4               # dst tiles per gather superbatch

BF16 = mybir.dt.bfloat16
F32 = mybir.dt.float32
I16 = mybir.dt.int16

LAST_EXEC_NS = None  # set when BASS_TRACE=1


# ---------------------------- host preprocessing ----------------------------

def _wrap_idx(flat):
    """int16 flat index list -> dma_gather wrapped layout [128, len/16]."""
    L = flat.shape[0]
    assert L % 16 == 0
    w = flat.reshape(L // 16, 16).T  # [16, W]
    return np.tile(w, (8, 1)).copy()  # [128, W]


def _prep(x, w1i, w1r, b1, w2i, w2r, b2, edge_index):
    src = np.asarray(edge_index[0], np.int64)
    dst = np.asarray(edge_index[1], np.int64)

    deg = np.bincount(dst, minlength=N).astype(np.float32)
    dinv = np.where(deg > 0, 1.0 / np.sqrt(np.maximum(deg, 1.0)), 0.0).astype(
        np.float32
    )
    dinv_pad = np.zeros(NPAD, np.float32)
    dinv_pad[:N] = dinv

    tile_g = dst // P          # global dst tile 0..319
    half = (src >= HALF).astype(np.int64)

    cnt = np.zeros((NCORES * TPC, 2), np.int64)
    np.add.at(cnt, (tile_g, half), 1)
    cnt3 = cnt.reshape(NCORES, TPC, 2)
    CH = np.ceil(cnt3.max(axis=0) / P).astype(np.int64)  # [TPC, 2] uniform
    CHA, CHB = CH[:, 0], CH[:, 1]
    CT = CHA + CHB
    colbase = np.concatenate([[0], np.cumsum(CT)])       # [TPC+1]
    TOTCH = int(colbase[-1])
    offA = np.concatenate([[0], np.cumsum(CHA)])         # chunks
    offB = np.concatenate([[0], np.cumsum(CHB)])
    LA, LB = int(offA[-1]) * P, int(offB[-1]) * P

    grp = tile_g * 2 + half
    order = np.argsort(grp, kind="stable")
    gs = grp[order]
    ss = src[order]
    ds = dst[order]
    gcnt = np.bincount(grp, minlength=NCORES * TPC * 2)
    gstart = np.concatenate([[0], np.cumsum(gcnt)])[:-1]
    pos = np.arange(E, dtype=np.int64) - gstart[gs]
    u = pos // P
    e = pos % P
    tg = gs // 2
    h = gs & 1
    core = tg // TPC
    tp = tg % TPC

    sl = (ss - h * HALF).astype(np.int16)
    gA = np.zeros((NCORES, max(LA, 16)), np.int16)
    gB = np.zeros((NCORES, max(LB, 16)), np.int16)
    mA = h == 0
    mB = ~mA
    flatA = (offA[tp[mA]] + u[mA]) * P + e[mA]
    flatB = (offB[tp[mB]] + u[mB]) * P + e[mB]
    gA[core[mA], flatA] = sl[mA]
    gB[core[mB], flatB] = sl[mB]

    dloc = (ds - tg * P).astype(np.float32)
    col = colbase[tp] + u + h * CHA[tp]
    dstl = np.full((NCORES, P, TOTCH), -1.0, np.float32)
    dstl[core, e, col] = dloc
    dstl = dstl.astype(ml_dtypes.bfloat16)

    # constants
    maxct = int(CT.max())
    iota = np.tile(np.arange(P, dtype=np.float32), maxct)
    iota = np.tile(iota[None, :], (P, 1)).astype(ml_dtypes.bfloat16)  # [128, maxct*128]
    ident = np.eye(P, dtype=ml_dtypes.bfloat16)

    # per-core tensors
    xpad = np.zeros((NPAD, F_IN), np.float32)
    xpad[:N] = x
    xT = np.ascontiguousarray(xpad.T)  # [512, 40960]

    in_maps = []
    for c in range(NCORES):
        xT_c = xT[:, c * NPC : (c + 1) * NPC].astype(ml_dtypes.bfloat16)
        dinv_c = dinv_pad[c * NPC : (c + 1) * NPC].reshape(TPC, P).T.copy()  # [128,40]
        in_maps.append(
            {
                "xT": np.ascontiguousarray(xT_c),
                "W1i": w1i.astype(ml_dtypes.bfloat16),
                "W1r": w1r.astype(ml_dtypes.bfloat16),
                "W2i": w2i.astype(ml_dtypes.bfloat16),
                "W2r": w2r.astype(ml_dtypes.bfloat16),
                "b1": b1.reshape(1, HID).astype(ml_dtypes.bfloat16),
                "b2": b2.reshape(1, NCLS).astype(ml_dtypes.bfloat16),
                "ones1": np.ones((1, P), ml_dtypes.bfloat16),
                "ident": ident,
                "iota": iota,
                "dinv": dinv_c,
                "dstl": np.ascontiguousarray(dstl[c]),
                "gidxA": _wrap_idx(gA[c]),
                "gidxB": _wrap_idx(gB[c]),
            }
        )

    meta = (tuple(int(v) for v in CHA), tuple(int(v) for v in CHB))
    return in_maps, meta


# ------------------------------ program build -------------------------------

_PROG_CACHE = {}


def _build(meta):
    if meta in _PROG_CACHE:
        return _PROG_CACHE[meta]

    CHA = np.array(meta[0])
    CHB = np.array(meta[1])
    CT = CHA + CHB
    colbase = np.concatenate([[0], np.cumsum(CT)])
    offA = np.concatenate([[0], np.cumsum(CHA)])
    offB = np.concatenate([[0], np.cumsum(CHB)])
    TOTCH = int(colbase[-1])
    maxct = int(CT.max())
    LA, LB = int(offA[-1]) * P, int(offB[-1]) * P
    nbatch = (TPC + SB - 1) // SB
    batches = [list(range(b * SB, min((b + 1) * SB, TPC))) for b in range(nbatch)]
    maxchA = max(int(CHA[b].sum()) for b in batches)
    maxchB = max(int(CHB[b].sum()) for b in batches)

    nc = bacc.Bacc("TRN2", target_bir_lowering=False, debug=False, num_devices=NCORES)

    xT = nc.dram_tensor("xT", [F_IN, NPC], BF16, kind="ExternalInput")
    W1i = nc.dram_tensor("W1i", [F_IN, HID], BF16, kind="ExternalInput")
    W1r = nc.dram_tensor("W1r", [F_IN, HID], BF16, kind="ExternalInput")
    W2i = nc.dram_tensor("W2i", [HID, NCLS], BF16, kind="ExternalInput")
    W2r = nc.dram_tensor("W2r", [HID, NCLS], BF16, kind="ExternalInput")
    b1 = nc.dram_tensor("b1", [1, HID], BF16, kind="ExternalInput")
    b2 = nc.dram_tensor("b2", [1, NCLS], BF16, kind="ExternalInput")
    ones1 = nc.dram_tensor("ones1", [1, P], BF16, kind="ExternalInput")
    ident = nc.dram_tensor("ident", [P, P], BF16, kind="ExternalInput")
    iota = nc.dram_tensor("iota", [P, maxct * P], BF16, kind="ExternalInput")
    dinv = nc.dram_tensor("dinv", [P, TPC], F32, kind="ExternalInput")
    dstl = nc.dram_tensor("dstl", [P, TOTCH], BF16, kind="ExternalInput")
    gidxA = nc.dram_tensor("gidxA", [P, max(LA, 16) // 16], I16, kind="ExternalInput")
    gidxB = nc.dram_tensor("gidxB", [P, max(LB, 16) // 16], I16, kind="ExternalInput")
    out_d = nc.dram_tensor("out", [NPC, NCLS], F32, kind="ExternalOutput")

    RELU = mybir.ActivationFunctionType.Relu
    COPY = mybir.ActivationFunctionType.Copy
    EQ = mybir.AluOpType.is_equal
    ADD = mybir.AluOpType.add

    with tile.TileContext(nc) as tc:
        with (
            tc.tile_pool(name="const", bufs=1) as constp,
            tc.tile_pool(name="sbuf", bufs=3) as sbuf,
            tc.tile_pool(name="big", bufs=1) as bigp,
            tc.tile_pool(name="msgs", bufs=2) as msgp,
            tc.tile_pool(name="psumd", bufs=4, space="PSUM") as psumd,
            tc.tile_pool(name="psuma", bufs=2, space="PSUM") as psuma,
            tc.tile_pool(name="psumt", bufs=2, space="PSUM") as psumt,
            tc.tile_pool(name="dram", bufs=1, space="DRAM") as dram,
        ):
            # ---- constants to SBUF ----
            w1i_t = constp.tile([P, KT, HID], BF16)
            w1r_t = constp.tile([P, KT, HID], BF16)
            for k in range(KT):
                nc.sync.dma_start(out=w1i_t[:, k, :], in_=W1i[k * P : (k + 1) * P, :])
                nc.sync.dma_start(out=w1r_t[:, k, :], in_=W1r[k * P : (k + 1) * P, :])
            w2i_t = constp.tile([P, NCLS], BF16)
            w2r_t = constp.tile([P, NCLS], BF16)
            nc.sync.dma_start(out=w2i_t[:], in_=W2i[:, :])
            nc.sync.dma_start(out=w2r_t[:], in_=W2r[:, :])
            b1_t = constp.tile([1, HID], BF16)
            b2_t = constp.tile([1, NCLS], BF16)
            ones_t = constp.tile([1, P], BF16)
            nc.sync.dma_start(out=b1_t[:], in_=b1[:, :])
            nc.sync.dma_start(out=b2_t[:], in_=b2[:, :])
            nc.sync.dma_start(out=ones_t[:], in_=ones1[:, :])
            ident_t = constp.tile([P, P], BF16)
            nc.sync.dma_start(out=ident_t[:], in_=ident[:, :])
            iota_t = constp.tile([P, maxct * P], BF16)
            nc.sync.dma_start(out=iota_t[:], in_=iota[:, :])
            dinv_t = constp.tile([P, TPC], F32)
            nc.sync.dma_start(out=dinv_t[:], in_=dinv[:, :])
            dstl_t = constp.tile([P, TOTCH], BF16)
            nc.sync.dma_start(out=dstl_t[:], in_=dstl[:, :])
            gA_t = constp.tile([P, max(LA, 16) // 16], I16)
            gB_t = constp.tile([P, max(LB, 16) // 16], I16)
            nc.sync.dma_start(out=gA_t[:], in_=gidxA[:, :])
            nc.sync.dma_start(out=gB_t[:], in_=gidxB[:, :])

            # persistent per-layer SBUF
            root1_t = bigp.tile([P, TPC, HID], BF16)
            out1T_t = bigp.tile([P, TPC, HID], BF16)
            root2_t = bigp.tile([P, TPC, NCLS], BF16)

            ag1_in = dram.tile([NPC, HID], BF16)
            ag1_out = dram.tile([NPAD, HID], BF16, addr_space="Shared")
            ag2_in = dram.tile([NPC, P], BF16)
            ag2_out = dram.tile([NPAD, P], BF16, addr_space="Shared")

            # ---- phase 1: dense layer 1 ----
            for b in range(TPC):
                xb = sbuf.tile([P, KT, P], BF16, tag="xb")
                for k in range(KT):
                    nc.sync.dma_start(
                        out=xb[:, k, :],
                        in_=xT[k * P : (k + 1) * P, b * P : (b + 1) * P],
                    )
                ps_i = psumd.tile([P, HID], F32, tag="d")
                ps_r = psumd.tile([P, HID], F32, tag="d")
                for k in range(KT):
                    nc.tensor.matmul(
                        ps_i[:], lhsT=xb[:, k, :], rhs=w1i_t[:, k, :],
                        start=(k == 0), stop=(k == KT - 1),
                    )
                for k in range(KT):
                    nc.tensor.matmul(
                        ps_r[:], lhsT=xb[:, k, :], rhs=w1r_t[:, k, :],
                        start=(k == 0), stop=False,
                    )
                nc.tensor.matmul(ps_r[:], lhsT=ones_t[:], rhs=b1_t[:],
                                 start=False, stop=True)
                hb = sbuf.tile([P, HID], BF16, tag="hb")
                nc.scalar.activation(hb[:], ps_i[:], COPY,
                                     scale=dinv_t[:, b : b + 1])
                nc.scalar.activation(root1_t[:, b, :], ps_r[:], COPY)
                nc.sync.dma_start(out=ag1_in[b * P : (b + 1) * P, :], in_=hb[:])

            # ---- phase 2: allgather h' ----
            nc.gpsimd.collective_compute(
                "AllGather",
                mybir.AluOpType.bypass,
                replica_groups=[list(range(NCORES))],
                ins=[ag1_in.opt()],
                outs=[ag1_out.opt()],
            )

            # ---- edge phase helper ----
            def edge_phase(table, fdim, root_t, w_i, w_r, bias_t, is_l2):
                for bi, bt in enumerate(batches):
                    nA = int(CHA[bt].sum())
                    nB = int(CHB[bt].sum())
                    mA_t = msgp.tile([P, max(maxchA, 1), P], BF16, tag="mA")
                    mB_t = msgp.tile([P, max(maxchB, 1), P], BF16, tag="mB")
                    if nA:
                        a0 = int(offA[bt[0]])
                        nc.gpsimd.dma_gather(
                            out_ap=mA_t[:, :nA, :],
                            in_ap=table[0:HALF, :],
                            idxs_ap=gA_t[:, a0 * 8 : (a0 + nA) * 8],
                            num_idxs=nA * P,
                            num_idxs_reg=nA * P,
                            elem_size=P,
                            single_packet=False,
                        )
                    if nB:
                        b0 = int(offB[bt[0]])
                        nc.gpsimd.dma_gather(
                            out_ap=mB_t[:, :nB, :],
                            in_ap=table[HALF:NPAD, :],
                            idxs_ap=gB_t[:, b0 * 8 : (b0 + nB) * 8],
                            num_idxs=nB * P,
                            num_idxs_reg=nB * P,
                            elem_size=P,
                            single_packet=False,
                        )
                    aoff = boff = 0
                    for t in bt:
                        ct = int(CT[t])
                        ca, cb = int(CHA[t]), int(CHB[t])
                        if ct == 0:
                            continue
                        s_t = sbuf.tile([P, maxct, P], BF16, tag="s")
                        nc.vector.tensor_tensor(
                            out=s_t[:, :ct, :],
                            in0=iota_t[:, : ct * P],
                            in1=dstl_t[:, colbase[t] : colbase[t] + ct].to_broadcast(
                                [P, ct, P]
                            ),
                            op=EQ,
                        )
                        ps_a = psuma.tile([P, fdim], F32, tag="a")
                        for u in range(ct):
                            if u < ca:
                                rhs = mA_t[:, aoff + u, :fdim]
                            else:
                                rhs = mB_t[:, boff + (u - ca), :fdim]
                            nc.tensor.matmul(
                                ps_a[:], lhsT=s_t[:, u, :], rhs=rhs,
                                start=(u == 0), stop=(u == ct - 1),
                            )
                        aoff += ca
                        boff += cb
                        # epilogue: relu(dinv*agg + root)
                        tt = sbuf.tile([P, fdim], BF16, tag="tt")
                        nc.scalar.activation(tt[:], ps_a[:], COPY,
                                             scale=dinv_t[:, t : t + 1])
                        o_t = sbuf.tile([P, fdim], F32 if is_l2 else BF16, tag="o")
                        nc.vector.tensor_tensor(
                            out=o_t[:], in0=tt[:], in1=root_t[:, t, :], op=ADD
                        )
                        if is_l2:
                            nc.scalar.activation(o_t[:], o_t[:], RELU)
                            nc.sync.dma_start(
                                out=out_d[t * P : (t + 1) * P, :], in_=o_t[:]
                            )
                        else:
                            nc.vector.tensor_scalar_max(o_t[:], o_t[:], 0.0)
                            # transpose for layer-2 dense
                            ps_t = psumt.tile([P, P], BF16, tag="t")
                            nc.tensor.transpose(ps_t[:], o_t[:], ident_t[:])
                            nc.scalar.activation(out1T_t[:, t, :], ps_t[:], COPY)
                            # layer-2 dense for this tile
                            ps_h2 = psumd.tile([P, NCLS], F32, tag="d")
                            ps_r2 = psumd.tile([P, NCLS], F32, tag="d")
                            nc.tensor.matmul(
                                ps_h2[:], lhsT=out1T_t[:, t, :], rhs=w_i[:],
                                start=True, stop=True,
                            )
                            nc.tensor.matmul(
                                ps_r2[:], lhsT=out1T_t[:, t, :], rhs=w_r[:],
                                start=True, stop=False,
                            )
                            nc.tensor.matmul(
                                ps_r2[:], lhsT=ones_t[:], rhs=bias_t[:],
                                start=False, stop=True,
                            )
                            h2b = sbuf.tile([P, NCLS], BF16, tag="h2b")
                            nc.scalar.activation(h2b[:], ps_h2[:], COPY,
                                                 scale=dinv_t[:, t : t + 1])
                            nc.scalar.activation(root2_t[:, t, :], ps_r2[:], COPY)
                            nc.sync.dma_start(
                                out=ag2_in[t * P : (t + 1) * P, :NCLS], in_=h2b[:]
                            )

            # ---- phase 3: edges layer 1 (+ fused dense layer 2) ----
            edge_phase(ag1_out, HID, root1_t, w2i_t, w2r_t, b2_t, is_l2=False)

            # ---- phase 4: allgather h2' ----
            nc.gpsimd.collective_compute(
                "AllGather",
                mybir.AluOpType.bypass,
                replica_groups=[list(range(NCORES))],
                ins=[ag2_in.opt()],
                outs=[ag2_out.opt()],
            )

            # ---- phase 5: edges layer 2 ----
            edge_phase(ag2_out, NCLS, root2_t, None, None, None, is_l2=True)

    nc.compile()
    _PROG_CACHE[meta] = nc
    return nc


# --------------------------------- kernel -----------------------------------

def kernel(**inputs):
    global LAST_EXEC_NS
    x = np.asarray(inputs["x"], np.float32)
    w1i = np.asarray(inputs["W1_init"], np.float32)
    w1r = np.asarray(inputs["W1_root"], np.float32)
    b1 = np.asarray(inputs["b1"], np.float32)
    w2i = np.asarray(inputs["W2_init"], np.float32)
    w2r = np.asarray(inputs["W2_root"], np.float32)
    b2 = np.asarray(inputs["b2"], np.float32)
    ei = np.asarray(inputs["edge_index"])

    in_maps, meta = _prep(x, w1i, w1r, b1, w2i, w2r, b2, ei)
    nc = _build(meta)

    trace = bool(int(os.environ.get("BASS_TRACE_KERNEL", "0")))
    r = run_bass_kernel_spmd(nc, in_maps, core_ids=list(range(NCORES)), trace=trace)
    if trace:
        LAST_EXEC_NS = r.exec_time_ns

    out = np.concatenate([r.results[c]["out"] for c in range(NCORES)], axis=0)
    return np.ascontiguousarray(out[:N]).astype(np.float32)


# revision 5
# speedup vs baseline: 1.5085x; 1.5085x over previous
"""Trainium2 Bass kernel for 2-layer ARMA GCN (nn_Net_33586644255234).

Strategy (graph/data parallel over 8 NeuronCores):
  - Nodes padded 40000 -> 40960 and sharded 5120/core (40 tiles of 128).
  - Weights replicated; per-core x^T shard shipped pre-transposed bf16.
  - Per layer:
      h' = dinv * (x @ W_init)          (dense, PE; dinv scale fused on ACT)
      AllGather h' -> full node table in every core's HBM
      per dst-tile: gather edge messages h'[src] with gpsimd.dma_gather,
      scatter-add via one-hot matmul:  psum += S_chunk^T @ msgs_chunk
      out = relu(dinv * psum + x @ W_root + b)
  - Edge bookkeeping (sort by dst tile, split by src half for int16 gather
    indices, chunk grids uniform across cores) is host-side sharding prep.

kernel(**inputs) takes FULL inputs, returns FULL [40000, 64] float32.
"""

import os
import sys

sys.path.insert(0, "/opt/trn_rl_repo")

import numpy as np
import ml_dtypes

import concourse.bass as bass
import concourse.mybir as mybir
import concourse.tile as tile
from concourse import bacc
from concourse.bass_utils import run_bass_kernel_spmd

# ---------------- problem constants (hardcoded per contract) ----------------
N, E, F_IN, HID, NCLS = 40000, 640000, 512, 128, 64
P = 128
NCORES = 8
NPC = 5120          # nodes per core (padded)
NPAD = NCORES * NPC  # 40960
TPC = NPC // P       # 40 dst tiles per core
KT = F_IN // P       # 4 k-tiles for layer-1 dense
HALF = NPAD // 2     # 20480 (int16-safe gather table half)
SB = 4               # dst tiles per gather superbatch

BF16 = mybir.dt.bfloat16
F32 = mybir.dt.float32
I16 = mybir.dt.int16

LAST_EXEC_NS = None  # set when BASS_TRACE=1


# ---------------------------- host preprocessing ----------------------------

def _wrap_idx(flat):
    """int16 flat index list -> dma_gather wrapped layout [128, len/16]."""
    L = flat.shape[0]
    assert L % 16 == 0
    w = flat.reshape(L // 16, 16).T  # [16, W]
    return np.tile(w, (8, 1)).copy()  # [128, W]


def _prep(x, w1i, w1r, b1, w2i, w2r, b2, edge_index):
    src = np.asarray(edge_index[0], np.int64)
    dst = np.asarray(edge_index[1], np.int64)

    deg = np.bincount(dst, minlength=N).astype(np.float32)
    dinv = np.where(deg > 0, 1.0 / np.sqrt(np.maximum(deg, 1.0)), 0.0).astype(
        np.float32
    )
    dinv_pad = np.zeros(NPAD, np.float32)
    dinv_pad[:N] = dinv

    tile_g = dst // P          # global dst tile 0..319
    half = (src >= HALF).astype(np.int64)

    cnt = np.zeros((NCORES * TPC, 2), np.int64)
    np.add.at(cnt, (tile_g, half), 1)
    cnt3 = cnt.reshape(NCORES, TPC, 2)
    CH = np.ceil(cnt3.max(axis=0) / P).astype(np.int64)  # [TPC, 2] uniform
    CHA, CHB = CH[:, 0], CH[:, 1]
    CT = CHA + CHB
    colbase = np.concatenate([[0], np.cumsum(CT)])       # [TPC+1]
    TOTCH = int(colbase[-1])
    offA = np.concatenate([[0], np.cumsum(CHA)])         # chunks
    offB = np.concatenate([[0], np.cumsum(CHB)])
    LA, LB = int(offA[-1]) * P, int(offB[-1]) * P

    grp = tile_g * 2 + half
    order = np.argsort(grp, kind="stable")
    gs = grp[order]
    ss = src[order]
    ds = dst[order]
    gcnt = np.bincount(grp, minlength=NCORES * TPC * 2)
    gstart = np.concatenate([[0], np.cumsum(gcnt)])[:-1]
    pos = np.arange(E, dtype=np.int64) - gstart[gs]
    u = pos // P
    e = pos % P
    tg = gs // 2
    h = gs & 1
    core = tg // TPC
    tp = tg % TPC

    sl = (ss - h * HALF).astype(np.int16)
    gA = np.zeros((NCORES, max(LA, 16)), np.int16)
    gB = np.zeros((NCORES, max(LB, 16)), np.int16)
    mA = h == 0
    mB = ~mA
    flatA = (offA[tp[mA]] + u[mA]) * P + e[mA]
    flatB = (offB[tp[mB]] + u[mB]) * P + e[mB]
    gA[core[mA], flatA] = sl[mA]
    gB[core[mB], flatB] = sl[mB]

    dloc = (ds - tg * P).astype(np.float32)
    col = colbase[tp] + u + h * CHA[tp]
    dstl = np.full((NCORES, P, TOTCH), -1.0, np.float32)
    dstl[core, e, col] = dloc
    dstl = dstl.astype(ml_dtypes.bfloat16)

    # constants
    maxct = int(CT.max())
    iota = np.tile(np.arange(P, dtype=np.float32), maxct)
    iota = np.tile(iota[None, :], (P, 1)).astype(ml_dtypes.bfloat16)  # [128, maxct*128]
    ident = np.eye(P, dtype=ml_dtypes.bfloat16)

    # per-core tensors
    xpad = np.zeros((NPAD, F_IN), np.float32)
    xpad[:N] = x
    xT = np.ascontiguousarray(xpad.T)  # [512, 40960]

    in_maps = []
    for c in range(NCORES):
        xT_c = xT[:, c * NPC : (c + 1) * NPC].astype(ml_dtypes.bfloat16)
        dinv_c = dinv_pad[c * NPC : (c + 1) * NPC].reshape(TPC, P).T.copy()  # [128,40]
        in_maps.append(
            {
                "xT": np.ascontiguousarray(xT_c),
                "W1i": w1i.astype(ml_dtypes.bfloat16),
                "W1r": w1r.astype(ml_dtypes.bfloat16),
                "W2i": w2i.astype(ml_dtypes.bfloat16),
                "W2r": w2r.astype(ml_dtypes.bfloat16),
                "b1": b1.reshape(1, HID).astype(ml_dtypes.bfloat16),
                "b2": b2.reshape(1, NCLS).astype(ml_dtypes.bfloat16),
                "ones1": np.ones((1, P), ml_dtypes.bfloat16),
                "ident": ident,
                "iota": iota,
                "dinv": dinv_c,
                "dstl": np.ascontiguousarray(dstl[c]),
                "gidxA": _wrap_idx(gA[c]),
                "gidxB": _wrap_idx(gB[c]),
            }
        )

    meta = (tuple(int(v) for v in CHA), tuple(int(v) for v in CHB))
    return in_maps, meta


# ------------------------------ program build -------------------------------

_PROG_CACHE = {}


def _build(meta):
    if meta in _PROG_CACHE:
        return _PROG_CACHE[meta]

    CHA = np.array(meta[0])
    CHB = np.array(meta[1])
    CT = CHA + CHB
    colbase = np.concatenate([[0], np.cumsum(CT)])
    offA = np.concatenate([[0], np.cumsum(CHA)])
    offB = np.concatenate([[0], np.cumsum(CHB)])
    TOTCH = int(colbase[-1])
    maxct = int(CT.max())
    LA, LB = int(offA[-1]) * P, int(offB[-1]) * P
    nbatch = (TPC + SB - 1) // SB
    batches = [list(range(b * SB, min((b + 1) * SB, TPC))) for b in range(nbatch)]
    maxchA = max(int(CHA[b].sum()) for b in batches)
    maxchB = max(int(CHB[b].sum()) for b in batches)

    nc = bacc.Bacc("TRN2", target_bir_lowering=False, debug=False, num_devices=NCORES, num_swdge_queues=4)

    xT = nc.dram_tensor("xT", [F_IN, NPC], BF16, kind="ExternalInput")
    W1i = nc.dram_tensor("W1i", [F_IN, HID], BF16, kind="ExternalInput")
    W1r = nc.dram_tensor("W1r", [F_IN, HID], BF16, kind="ExternalInput")
    W2i = nc.dram_tensor("W2i", [HID, NCLS], BF16, kind="ExternalInput")
    W2r = nc.dram_tensor("W2r", [HID, NCLS], BF16, kind="ExternalInput")
    b1 = nc.dram_tensor("b1", [1, HID], BF16, kind="ExternalInput")
    b2 = nc.dram_tensor("b2", [1, NCLS], BF16, kind="ExternalInput")
    ones1 = nc.dram_tensor("ones1", [1, P], BF16, kind="ExternalInput")
    ident = nc.dram_tensor("ident", [P, P], BF16, kind="ExternalInput")
    iota = nc.dram_tensor("iota", [P, maxct * P], BF16, kind="ExternalInput")
    dinv = nc.dram_tensor("dinv", [P, TPC], F32, kind="ExternalInput")
    dstl = nc.dram_tensor("dstl", [P, TOTCH], BF16, kind="ExternalInput")
    gidxA = nc.dram_tensor("gidxA", [P, max(LA, 16) // 16], I16, kind="ExternalInput")
    gidxB = nc.dram_tensor("gidxB", [P, max(LB, 16) // 16], I16, kind="ExternalInput")
    out_d = nc.dram_tensor("out", [NPC, NCLS], F32, kind="ExternalOutput")

    RELU = mybir.ActivationFunctionType.Relu
    COPY = mybir.ActivationFunctionType.Copy
    EQ = mybir.AluOpType.is_equal
    ADD = mybir.AluOpType.add

    with tile.TileContext(nc) as tc:
        with (
            tc.tile_pool(name="const", bufs=1) as constp,
            tc.tile_pool(name="sbuf", bufs=3) as sbuf,
            tc.tile_pool(name="big", bufs=1) as bigp,
            tc.tile_pool(name="msgs", bufs=2) as msgp,
            tc.tile_pool(name="psumd", bufs=4, space="PSUM") as psumd,
            tc.tile_pool(name="psuma", bufs=2, space="PSUM") as psuma,
            tc.tile_pool(name="psumt", bufs=2, space="PSUM") as psumt,
            tc.tile_pool(name="dram", bufs=1, space="DRAM") as dram,
        ):
            # ---- constants to SBUF ----
            w1i_t = constp.tile([P, KT, HID], BF16)
            w1r_t = constp.tile([P, KT, HID], BF16)
            for k in range(KT):
                nc.sync.dma_start(out=w1i_t[:, k, :], in_=W1i[k * P : (k + 1) * P, :])
                nc.sync.dma_start(out=w1r_t[:, k, :], in_=W1r[k * P : (k + 1) * P, :])
            w2i_t = constp.tile([P, NCLS], BF16)
            w2r_t = constp.tile([P, NCLS], BF16)
            nc.sync.dma_start(out=w2i_t[:], in_=W2i[:, :])
            nc.sync.dma_start(out=w2r_t[:], in_=W2r[:, :])
            b1_t = constp.tile([1, HID], BF16)
            b2_t = constp.tile([1, NCLS], BF16)
            ones_t = constp.tile([1, P], BF16)
            nc.sync.dma_start(out=b1_t[:], in_=b1[:, :])
            nc.sync.dma_start(out=b2_t[:], in_=b2[:, :])
            nc.sync.dma_start(out=ones_t[:], in_=ones1[:, :])
            ident_t = constp.tile([P, P], BF16)
            nc.sync.dma_start(out=ident_t[:], in_=ident[:, :])
            iota_t = constp.tile([P, maxct * P], BF16)
            nc.sync.dma_start(out=iota_t[:], in_=iota[:, :])
            dinv_t = constp.tile([P, TPC], F32)
            nc.sync.dma_start(out=dinv_t[:], in_=dinv[:, :])
            dstl_t = constp.tile([P, TOTCH], BF16)
            nc.sync.dma_start(out=dstl_t[:], in_=dstl[:, :])
            gA_t = constp.tile([P, max(LA, 16) // 16], I16)
            gB_t = constp.tile([P, max(LB, 16) // 16], I16)
            nc.sync.dma_start(out=gA_t[:], in_=gidxA[:, :])
            nc.sync.dma_start(out=gB_t[:], in_=gidxB[:, :])

            # persistent per-layer SBUF
            root1_t = bigp.tile([P, TPC, HID], BF16)
            out1T_t = bigp.tile([P, TPC, HID], BF16)
            root2_t = bigp.tile([P, TPC, NCLS], BF16)

            ag1_in = dram.tile([NPC, HID], BF16)
            ag1_out = dram.tile([NPAD, HID], BF16, addr_space="Shared")
            ag2_in = dram.tile([NPC, P], BF16)
            ag2_out = dram.tile([NPAD, P], BF16, addr_space="Shared")

            # ---- phase 1: dense layer 1 ----
            for b in range(TPC):
                xb = sbuf.tile([P, KT, P], BF16, tag="xb")
                for k in range(KT):
                    nc.sync.dma_start(
                        out=xb[:, k, :],
                        in_=xT[k * P : (k + 1) * P, b * P : (b + 1) * P],
                    )
                ps_i = psumd.tile([P, HID], F32, tag="d")
                ps_r = psumd.tile([P, HID], F32, tag="d")
                for k in range(KT):
                    nc.tensor.matmul(
                        ps_i[:], lhsT=xb[:, k, :], rhs=w1i_t[:, k, :],
                        start=(k == 0), stop=(k == KT - 1),
                    )
                for k in range(KT):
                    nc.tensor.matmul(
                        ps_r[:], lhsT=xb[:, k, :], rhs=w1r_t[:, k, :],
                        start=(k == 0), stop=False,
                    )
                nc.tensor.matmul(ps_r[:], lhsT=ones_t[:], rhs=b1_t[:],
                                 start=False, stop=True)
                hb = sbuf.tile([P, HID], BF16, tag="hb")
                nc.scalar.activation(hb[:], ps_i[:], COPY,
                                     scale=dinv_t[:, b : b + 1])
                nc.scalar.activation(root1_t[:, b, :], ps_r[:], COPY)
                nc.sync.dma_start(out=ag1_in[b * P : (b + 1) * P, :], in_=hb[:])

            # ---- phase 2: allgather h' ----
            nc.gpsimd.collective_compute(
                "AllGather",
                mybir.AluOpType.bypass,
                replica_groups=[list(range(NCORES))],
                ins=[ag1_in.opt()],
                outs=[ag1_out.opt()],
            )

            # ---- edge phase helper ----
            def edge_phase(table, fdim, root_t, w_i, w_r, bias_t, is_l2):
                for bi, bt in enumerate(batches):
                    nA = int(CHA[bt].sum())
                    nB = int(CHB[bt].sum())
                    qA = (2 * bi) % 4
                    qB = (2 * bi + 1) % 4
                    mA_t = msgp.tile([P, max(maxchA, 1), P], BF16, tag="mA")
                    mB_t = msgp.tile([P, max(maxchB, 1), P], BF16, tag="mB")
                    if nA:
                        a0 = int(offA[bt[0]])
                        nc.gpsimd.dma_gather(
                            out_ap=mA_t[:, :nA, :],
                            in_ap=table[0:HALF, :],
                            idxs_ap=gA_t[:, a0 * 8 : (a0 + nA) * 8],
                            num_idxs=nA * P,
                            num_idxs_reg=nA * P,
                            elem_size=P,
                            single_packet=False,
                            queue_num=qA,
                        )
                    if nB:
                        b0 = int(offB[bt[0]])
                        nc.gpsimd.dma_gather(
                            out_ap=mB_t[:, :nB, :],
                            in_ap=table[HALF:NPAD, :],
                            idxs_ap=gB_t[:, b0 * 8 : (b0 + nB) * 8],
                            num_idxs=nB * P,
                            num_idxs_reg=nB * P,
                            elem_size=P,
                            single_packet=False,
                            queue_num=qB,
                        )
                    aoff = boff = 0
                    for t in bt:
                        ct = int(CT[t])
                        ca, cb = int(CHA[t]), int(CHB[t])
                        if ct == 0:
                            continue
                        s_t = sbuf.tile([P, maxct, P], BF16, tag="s")
                        nc.vector.tensor_tensor(
                            out=s_t[:, :ct, :],
                            in0=iota_t[:, : ct * P],
                            in1=dstl_t[:, colbase[t] : colbase[t] + ct].to_broadcast(
                                [P, ct, P]
                            ),
                            op=EQ,
                        )
                        ps_a = psuma.tile([P, fdim], F32, tag="a")
                        for u in range(ct):
                            if u < ca:
                                rhs = mA_t[:, aoff + u, :fdim]
                            else:
                                rhs = mB_t[:, boff + (u - ca), :fdim]
                            nc.tensor.matmul(
                                ps_a[:], lhsT=s_t[:, u, :], rhs=rhs,
                                start=(u == 0), stop=(u == ct - 1),
                            )
                        aoff += ca
                        boff += cb
                        # epilogue: relu(dinv*agg + root)
                        tt = sbuf.tile([P, fdim], BF16, tag="tt")
                        nc.scalar.activation(tt[:], ps_a[:], COPY,
                                             scale=dinv_t[:, t : t + 1])
                        o_pre = sbuf.tile([P, fdim], BF16, tag="opre")
                        nc.vector.tensor_tensor(
                            out=o_pre[:], in0=tt[:], in1=root_t[:, t, :], op=ADD
                        )
                        o_t = sbuf.tile([P, fdim], F32 if is_l2 else BF16, tag="o")
                        nc.scalar.activation(o_t[:], o_pre[:], RELU)
                        if is_l2:
                            nc.sync.dma_start(
                                out=out_d[t * P : (t + 1) * P, :], in_=o_t[:]
                            )
                        else:
                            # transpose for layer-2 dense
                            ps_t = psumt.tile([P, P], BF16, tag="t")
                            nc.tensor.transpose(ps_t[:], o_t[:], ident_t[:])
                            nc.scalar.activation(out1T_t[:, t, :], ps_t[:], COPY)
                            # layer-2 dense for this tile
                            ps_h2 = psumd.tile([P, NCLS], F32, tag="d")
                            ps_r2 = psumd.tile([P, NCLS], F32, tag="d")
                            nc.tensor.matmul(
                                ps_h2[:], lhsT=out1T_t[:, t, :], rhs=w_i[:],
                                start=True, stop=True,
                            )
                            nc.tensor.matmul(
                                ps_r2[:], lhsT=out1T_t[:, t, :], rhs=w_r[:],
                                start=True, stop=False,
                            )
                            nc.tensor.matmul(
                                ps_r2[:], lhsT=ones_t[:], rhs=bias_t[:],
                                start=False, stop=True,
                            )
                            h2b = sbuf.tile([P, NCLS], BF16, tag="h2b")
                            nc.scalar.activation(h2b[:], ps_h2[:], COPY,
                                                 scale=dinv_t[:, t : t + 1])
                            nc.scalar.activation(root2_t[:, t, :], ps_r2[:], COPY)
                            nc.sync.dma_start(
                                out=ag2_in[t * P : (t + 1) * P, :NCLS], in_=h2b[:]
                            )

            # ---- phase 3: edges layer 1 (+ fused dense layer 2) ----
            edge_phase(ag1_out, HID, root1_t, w2i_t, w2r_t, b2_t, is_l2=False)

            # ---- phase 4: allgather h2' ----
            nc.gpsimd.collective_compute(
                "AllGather",
                mybir.AluOpType.bypass,
                replica_groups=[list(range(NCORES))],
                ins=[ag2_in.opt()],
                outs=[ag2_out.opt()],
            )

            # ---- phase 5: edges layer 2 ----
            edge_phase(ag2_out, NCLS, root2_t, None, None, None, is_l2=True)

    nc.compile()
    _PROG_CACHE[meta] = nc
    return nc


# --------------------------------- kernel -----------------------------------

def kernel(**inputs):
    global LAST_EXEC_NS
    x = np.asarray(inputs["x"], np.float32)
    w1i = np.asarray(inputs["W1_init"], np.float32)
    w1r = np.asarray(inputs["W1_root"], np.float32)
    b1 = np.asarray(inputs["b1"], np.float32)
    w2i = np.asarray(inputs["W2_init"], np.float32)
    w2r = np.asarray(inputs["W2_root"], np.float32)
    b2 = np.asarray(inputs["b2"], np.float32)
    ei = np.asarray(inputs["edge_index"])

    in_maps, meta = _prep(x, w1i, w1r, b1, w2i, w2r, b2, ei)
    nc = _build(meta)

    trace = bool(int(os.environ.get("BASS_TRACE_KERNEL", "0")))
    r = run_bass_kernel_spmd(nc, in_maps, core_ids=list(range(NCORES)), trace=trace)
    if trace:
        LAST_EXEC_NS = r.exec_time_ns

    out = np.concatenate([r.results[c]["out"] for c in range(NCORES)], axis=0)
    return np.ascontiguousarray(out[:N]).astype(np.float32)


# revision 7
# speedup vs baseline: 1.7019x; 1.1282x over previous
"""Trainium2 Bass kernel for 2-layer ARMA GCN (nn_Net_33586644255234).

Strategy (graph/data parallel over 8 NeuronCores):
  - Nodes padded 40000 -> 40960 and sharded 5120/core (40 tiles of 128).
  - Weights replicated; per-core x^T shard shipped pre-transposed bf16.
  - Per layer:
      h' = dinv * (x @ W_init)          (dense, PE; dinv scale fused on ACT)
      AllGather h' -> full node table in every core's HBM
      per dst-tile: gather edge messages h'[src] with gpsimd.dma_gather,
      scatter-add via one-hot matmul:  psum += S_chunk^T @ msgs_chunk
      out = relu(dinv * psum + x @ W_root + b)
  - Edge bookkeeping (sort by dst tile, split by src half for int16 gather
    indices, chunk grids uniform across cores) is host-side sharding prep.

kernel(**inputs) takes FULL inputs, returns FULL [40000, 64] float32.
"""

import os
import sys

sys.path.insert(0, "/opt/trn_rl_repo")

import numpy as np
import ml_dtypes

import concourse.bass as bass
import concourse.mybir as mybir
import concourse.tile as tile
from concourse import bacc
from concourse.bass_utils import run_bass_kernel_spmd

# ---------------- problem constants (hardcoded per contract) ----------------
N, E, F_IN, HID, NCLS = 40000, 640000, 512, 128, 64
P = 128
NCORES = 8
NPC = 5120          # nodes per core (padded)
NPAD = NCORES * NPC  # 40960
TPC = NPC // P       # 40 dst tiles per core
KT = F_IN // P       # 4 k-tiles for layer-1 dense
HALF = NPAD // 2     # 20480 (int16-safe gather table half)
SB = 4               # dst tiles per gather superbatch

BF16 = mybir.dt.bfloat16
F32 = mybir.dt.float32
I16 = mybir.dt.int16

LAST_EXEC_NS = None  # set when BASS_TRACE=1


# ---------------------------- host preprocessing ----------------------------

def _wrap_idx(flat):
    """int16 flat index list -> dma_gather wrapped layout [128, len/16]."""
    L = flat.shape[0]
    assert L % 16 == 0
    w = flat.reshape(L // 16, 16).T  # [16, W]
    return np.tile(w, (8, 1)).copy()  # [128, W]


def _prep(x, w1i, w1r, b1, w2i, w2r, b2, edge_index):
    src = np.asarray(edge_index[0], np.int64)
    dst = np.asarray(edge_index[1], np.int64)

    deg = np.bincount(dst, minlength=N).astype(np.float32)
    dinv = np.where(deg > 0, 1.0 / np.sqrt(np.maximum(deg, 1.0)), 0.0).astype(
        np.float32
    )
    dinv_pad = np.zeros(NPAD, np.float32)
    dinv_pad[:N] = dinv

    tile_g = dst // P          # global dst tile 0..319
    s_rank = src // NPC
    s_q = src % NPC
    half = (s_q >= NPC // 2).astype(np.int64)
    s_local = (s_rank * (NPC // 2) + (s_q % (NPC // 2))).astype(np.int64)

    cnt = np.zeros((NCORES * TPC, 2), np.int64)
    np.add.at(cnt, (tile_g, half), 1)
    cnt3 = cnt.reshape(NCORES, TPC, 2)
    CH = np.ceil(cnt3.max(axis=0) / P).astype(np.int64)  # [TPC, 2] uniform
    CHA, CHB = CH[:, 0], CH[:, 1]
    CT = CHA + CHB
    colbase = np.concatenate([[0], np.cumsum(CT)])       # [TPC+1]
    TOTCH = int(colbase[-1])
    offA = np.concatenate([[0], np.cumsum(CHA)])         # chunks
    offB = np.concatenate([[0], np.cumsum(CHB)])
    LA, LB = int(offA[-1]) * P, int(offB[-1]) * P

    grp = tile_g * 2 + half
    order = np.argsort(grp, kind="stable")
    gs = grp[order]
    ss = src[order]
    ds = dst[order]
    gcnt = np.bincount(grp, minlength=NCORES * TPC * 2)
    gstart = np.concatenate([[0], np.cumsum(gcnt)])[:-1]
    pos = np.arange(E, dtype=np.int64) - gstart[gs]
    u = pos // P
    e = pos % P
    tg = gs // 2
    h = gs & 1
    core = tg // TPC
    tp = tg % TPC

    sl = s_local[order].astype(np.int16)
    gA = np.zeros((NCORES, max(LA, 16)), np.int16)
    gB = np.zeros((NCORES, max(LB, 16)), np.int16)
    mA = h == 0
    mB = ~mA
    flatA = (offA[tp[mA]] + u[mA]) * P + e[mA]
    flatB = (offB[tp[mB]] + u[mB]) * P + e[mB]
    gA[core[mA], flatA] = sl[mA]
    gB[core[mB], flatB] = sl[mB]

    dloc = (ds - tg * P).astype(np.float32)
    col = colbase[tp] + u + h * CHA[tp]
    dstl = np.full((NCORES, P, TOTCH), -1.0, np.float32)
    dstl[core, e, col] = dloc
    dstl = dstl.astype(ml_dtypes.bfloat16)

    # constants
    maxct = int(CT.max())
    iota = np.tile(np.arange(P, dtype=np.float32), maxct)
    iota = np.tile(iota[None, :], (P, 1)).astype(ml_dtypes.bfloat16)  # [128, maxct*128]
    ident = np.eye(P, dtype=ml_dtypes.bfloat16)

    # per-core tensors
    xpad = np.zeros((NPAD, F_IN), np.float32)
    xpad[:N] = x
    xT = np.ascontiguousarray(xpad.T)  # [512, 40960]

    in_maps = []
    for c in range(NCORES):
        xT_c = xT[:, c * NPC : (c + 1) * NPC].astype(ml_dtypes.bfloat16)
        dinv_c = dinv_pad[c * NPC : (c + 1) * NPC].reshape(TPC, P).T.copy()  # [128,40]
        in_maps.append(
            {
                "xT": np.ascontiguousarray(xT_c),
                "W1i": w1i.astype(ml_dtypes.bfloat16),
                "W1r": w1r.astype(ml_dtypes.bfloat16),
                "W2i": w2i.astype(ml_dtypes.bfloat16),
                "W2r": w2r.astype(ml_dtypes.bfloat16),
                "b1": b1.reshape(1, HID).astype(ml_dtypes.bfloat16),
                "b2": b2.reshape(1, NCLS).astype(ml_dtypes.bfloat16),
                "ones1": np.ones((1, P), ml_dtypes.bfloat16),
                "ident": ident,
                "iota": iota,
                "dinv": dinv_c,
                "dstl": np.ascontiguousarray(dstl[c]),
                "gidxA": _wrap_idx(gA[c]),
                "gidxB": _wrap_idx(gB[c]),
            }
        )

    meta = (tuple(int(v) for v in CHA), tuple(int(v) for v in CHB))
    return in_maps, meta


# ------------------------------ program build -------------------------------

_PROG_CACHE = {}


def _build(meta):
    if meta in _PROG_CACHE:
        return _PROG_CACHE[meta]

    CHA = np.array(meta[0])
    CHB = np.array(meta[1])
    CT = CHA + CHB
    colbase = np.concatenate([[0], np.cumsum(CT)])
    offA = np.concatenate([[0], np.cumsum(CHA)])
    offB = np.concatenate([[0], np.cumsum(CHB)])
    TOTCH = int(colbase[-1])
    maxct = int(CT.max())
    LA, LB = int(offA[-1]) * P, int(offB[-1]) * P
    nbatch = (TPC + SB - 1) // SB
    batches = [list(range(b * SB, min((b + 1) * SB, TPC))) for b in range(nbatch)]
    maxchA = max(int(CHA[b].sum()) for b in batches)
    maxchB = max(int(CHB[b].sum()) for b in batches)

    nc = bacc.Bacc("TRN2", target_bir_lowering=False, debug=False, num_devices=NCORES, num_swdge_queues=4)

    xT = nc.dram_tensor("xT", [F_IN, NPC], BF16, kind="ExternalInput")
    W1i = nc.dram_tensor("W1i", [F_IN, HID], BF16, kind="ExternalInput")
    W1r = nc.dram_tensor("W1r", [F_IN, HID], BF16, kind="ExternalInput")
    W2i = nc.dram_tensor("W2i", [HID, NCLS], BF16, kind="ExternalInput")
    W2r = nc.dram_tensor("W2r", [HID, NCLS], BF16, kind="ExternalInput")
    b1 = nc.dram_tensor("b1", [1, HID], BF16, kind="ExternalInput")
    b2 = nc.dram_tensor("b2", [1, NCLS], BF16, kind="ExternalInput")
    ones1 = nc.dram_tensor("ones1", [1, P], BF16, kind="ExternalInput")
    ident = nc.dram_tensor("ident", [P, P], BF16, kind="ExternalInput")
    iota = nc.dram_tensor("iota", [P, maxct * P], BF16, kind="ExternalInput")
    dinv = nc.dram_tensor("dinv", [P, TPC], F32, kind="ExternalInput")
    dstl = nc.dram_tensor("dstl", [P, TOTCH], BF16, kind="ExternalInput")
    gidxA = nc.dram_tensor("gidxA", [P, max(LA, 16) // 16], I16, kind="ExternalInput")
    gidxB = nc.dram_tensor("gidxB", [P, max(LB, 16) // 16], I16, kind="ExternalInput")
    out_d = nc.dram_tensor("out", [NPC, NCLS], F32, kind="ExternalOutput")

    RELU = mybir.ActivationFunctionType.Relu
    COPY = mybir.ActivationFunctionType.Copy
    EQ = mybir.AluOpType.is_equal
    ADD = mybir.AluOpType.add

    with tile.TileContext(nc) as tc:
        with (
            tc.tile_pool(name="const", bufs=1) as constp,
            tc.tile_pool(name="sbuf", bufs=3) as sbuf,
            tc.tile_pool(name="big", bufs=1) as bigp,
            tc.tile_pool(name="msgs", bufs=3) as msgp,
            tc.tile_pool(name="psumd", bufs=4, space="PSUM") as psumd,
            tc.tile_pool(name="psuma", bufs=2, space="PSUM") as psuma,
            tc.tile_pool(name="psumt", bufs=2, space="PSUM") as psumt,
            tc.tile_pool(name="dram", bufs=1, space="DRAM") as dram,
        ):
            # ---- constants to SBUF ----
            w1i_t = constp.tile([P, KT, HID], BF16)
            w1r_t = constp.tile([P, KT, HID], BF16)
            for k in range(KT):
                nc.sync.dma_start(out=w1i_t[:, k, :], in_=W1i[k * P : (k + 1) * P, :])
                nc.sync.dma_start(out=w1r_t[:, k, :], in_=W1r[k * P : (k + 1) * P, :])
            w2i_t = constp.tile([P, NCLS], BF16)
            w2r_t = constp.tile([P, NCLS], BF16)
            nc.sync.dma_start(out=w2i_t[:], in_=W2i[:, :])
            nc.sync.dma_start(out=w2r_t[:], in_=W2r[:, :])
            b1_t = constp.tile([1, HID], BF16)
            b2_t = constp.tile([1, NCLS], BF16)
            ones_t = constp.tile([1, P], BF16)
            nc.sync.dma_start(out=b1_t[:], in_=b1[:, :])
            nc.sync.dma_start(out=b2_t[:], in_=b2[:, :])
            nc.sync.dma_start(out=ones_t[:], in_=ones1[:, :])
            ident_t = constp.tile([P, P], BF16)
            nc.sync.dma_start(out=ident_t[:], in_=ident[:, :])
            iota_t = constp.tile([P, maxct * P], BF16)
            nc.sync.dma_start(out=iota_t[:], in_=iota[:, :])
            dinv_t = constp.tile([P, TPC], F32)
            nc.sync.dma_start(out=dinv_t[:], in_=dinv[:, :])
            dstl_t = constp.tile([P, TOTCH], BF16)
            nc.sync.dma_start(out=dstl_t[:], in_=dstl[:, :])
            gA_t = constp.tile([P, max(LA, 16) // 16], I16)
            gB_t = constp.tile([P, max(LB, 16) // 16], I16)
            nc.sync.dma_start(out=gA_t[:], in_=gidxA[:, :])
            nc.sync.dma_start(out=gB_t[:], in_=gidxB[:, :])

            # persistent per-layer SBUF
            root1_t = bigp.tile([P, TPC, HID], BF16)
            out1T_t = bigp.tile([P, TPC, HID], BF16)
            root2_t = bigp.tile([P, TPC, NCLS], BF16)

            HN = NPC // 2
            ag1_in0 = dram.tile([HN, HID], BF16)
            ag1_in1 = dram.tile([HN, HID], BF16)
            ag1_out0 = dram.tile([NPAD // 2, HID], BF16, addr_space="Shared")
            ag1_out1 = dram.tile([NPAD // 2, HID], BF16, addr_space="Shared")
            ag2_in0 = dram.tile([HN, P], BF16)
            ag2_in1 = dram.tile([HN, P], BF16)
            ag2_out0 = dram.tile([NPAD // 2, P], BF16, addr_space="Shared")
            ag2_out1 = dram.tile([NPAD // 2, P], BF16, addr_space="Shared")

            def do_ag(src, dst):
                nc.gpsimd.collective_compute(
                    "AllGather",
                    mybir.AluOpType.bypass,
                    replica_groups=[list(range(NCORES))],
                    ins=[src.opt()],
                    outs=[dst.opt()],
                )

            # ---- phase 1: dense layer 1 ----
            for b in range(TPC):
                xb = sbuf.tile([P, KT, P], BF16, tag="xb")
                for k in range(KT):
                    nc.sync.dma_start(
                        out=xb[:, k, :],
                        in_=xT[k * P : (k + 1) * P, b * P : (b + 1) * P],
                    )
                ps_i = psumd.tile([P, HID], F32, tag="d")
                ps_r = psumd.tile([P, HID], F32, tag="d")
                for k in range(KT):
                    nc.tensor.matmul(
                        ps_i[:], lhsT=xb[:, k, :], rhs=w1i_t[:, k, :],
                        start=(k == 0), stop=(k == KT - 1),
                    )
                for k in range(KT):
                    nc.tensor.matmul(
                        ps_r[:], lhsT=xb[:, k, :], rhs=w1r_t[:, k, :],
                        start=(k == 0), stop=False,
                    )
                nc.tensor.matmul(ps_r[:], lhsT=ones_t[:], rhs=b1_t[:],
                                 start=False, stop=True)
                hb = sbuf.tile([P, HID], BF16, tag="hb")
                nc.scalar.activation(hb[:], ps_i[:], COPY,
                                     scale=dinv_t[:, b : b + 1])
                nc.scalar.activation(root1_t[:, b, :], ps_r[:], COPY)
                if b < TPC // 2:
                    nc.sync.dma_start(out=ag1_in0[b * P : (b + 1) * P, :], in_=hb[:])
                else:
                    bb = b - TPC // 2
                    nc.sync.dma_start(out=ag1_in1[bb * P : (bb + 1) * P, :], in_=hb[:])
                if b == TPC // 2 - 1:
                    do_ag(ag1_in0, ag1_out0)
            # ---- phase 2: allgather h' (second half) ----
            do_ag(ag1_in1, ag1_out1)

            # ---- edge phase helper ----
            def edge_phase(tables, fdim, root_t, w_i, w_r, bias_t, is_l2):
                for bi, bt in enumerate(batches):
                    nA = int(CHA[bt].sum())
                    nB = int(CHB[bt].sum())
                    qA = (2 * bi) % 4
                    qB = (2 * bi + 1) % 4
                    mA_t = msgp.tile([P, max(maxchA, 1), P], BF16, tag="mA")
                    mB_t = msgp.tile([P, max(maxchB, 1), P], BF16, tag="mB")
                    if nA:
                        a0 = int(offA[bt[0]])
                        nc.gpsimd.dma_gather(
                            out_ap=mA_t[:, :nA, :],
                            in_ap=tables[0][:, :],
                            idxs_ap=gA_t[:, a0 * 8 : (a0 + nA) * 8],
                            num_idxs=nA * P,
                            num_idxs_reg=nA * P,
                            elem_size=P,
                            single_packet=False,
                            queue_num=qA,
                        )
                    if nB:
                        b0 = int(offB[bt[0]])
                        nc.gpsimd.dma_gather(
                            out_ap=mB_t[:, :nB, :],
                            in_ap=tables[1][:, :],
                            idxs_ap=gB_t[:, b0 * 8 : (b0 + nB) * 8],
                            num_idxs=nB * P,
                            num_idxs_reg=nB * P,
                            elem_size=P,
                            single_packet=False,
                            queue_num=qB,
                        )
                    aoff = boff = 0
                    for t in bt:
                        ct = int(CT[t])
                        ca, cb = int(CHA[t]), int(CHB[t])
                        if ct == 0:
                            continue
                        s_t = sbuf.tile([P, maxct, P], BF16, tag="s")
                        nc.vector.tensor_tensor(
                            out=s_t[:, :ct, :],
                            in0=iota_t[:, : ct * P],
                            in1=dstl_t[:, colbase[t] : colbase[t] + ct].to_broadcast(
                                [P, ct, P]
                            ),
                            op=EQ,
                        )
                        ps_a = psuma.tile([P, fdim], F32, tag="a")
                        for u in range(ct):
                            if u < ca:
                                rhs = mA_t[:, aoff + u, :fdim]
                            else:
                                rhs = mB_t[:, boff + (u - ca), :fdim]
                            nc.tensor.matmul(
                                ps_a[:], lhsT=s_t[:, u, :], rhs=rhs,
                                start=(u == 0), stop=(u == ct - 1),
                            )
                        aoff += ca
                        boff += cb
                        # epilogue: relu(dinv*agg + root)
                        tt = sbuf.tile([P, fdim], BF16, tag="tt")
                        nc.scalar.activation(tt[:], ps_a[:], COPY,
                                             scale=dinv_t[:, t : t + 1])
                        o_pre = sbuf.tile([P, fdim], BF16, tag="opre")
                        nc.vector.tensor_tensor(
                            out=o_pre[:], in0=tt[:], in1=root_t[:, t, :], op=ADD
                        )
                        o_t = sbuf.tile([P, fdim], F32 if is_l2 else BF16, tag="o")
                        nc.scalar.activation(o_t[:], o_pre[:], RELU)
                        if is_l2:
                            nc.sync.dma_start(
                                out=out_d[t * P : (t + 1) * P, :], in_=o_t[:]
                            )
                        else:
                            # transpose for layer-2 dense
                            ps_t = psumt.tile([P, P], BF16, tag="t")
                            nc.tensor.transpose(ps_t[:], o_t[:], ident_t[:])
                            nc.scalar.activation(out1T_t[:, t, :], ps_t[:], COPY)
                            # layer-2 dense for this tile
                            ps_h2 = psumd.tile([P, NCLS], F32, tag="d")
                            ps_r2 = psumd.tile([P, NCLS], F32, tag="d")
                            nc.tensor.matmul(
                                ps_h2[:], lhsT=out1T_t[:, t, :], rhs=w_i[:],
                                start=True, stop=True,
                            )
                            nc.tensor.matmul(
                                ps_r2[:], lhsT=out1T_t[:, t, :], rhs=w_r[:],
                                start=True, stop=False,
                            )
                            nc.tensor.matmul(
                                ps_r2[:], lhsT=ones_t[:], rhs=bias_t[:],
                                start=False, stop=True,
                            )
                            h2b = sbuf.tile([P, NCLS], BF16, tag="h2b")
                            nc.scalar.activation(h2b[:], ps_h2[:], COPY,
                                                 scale=dinv_t[:, t : t + 1])
                            nc.scalar.activation(root2_t[:, t, :], ps_r2[:], COPY)
                            if t < TPC // 2:
                                nc.sync.dma_start(
                                    out=ag2_in0[t * P : (t + 1) * P, :NCLS],
                                    in_=h2b[:],
                                )
                            else:
                                t2 = t - TPC // 2
                                nc.sync.dma_start(
                                    out=ag2_in1[t2 * P : (t2 + 1) * P, :NCLS],
                                    in_=h2b[:],
                                )
                            if t == TPC // 2 - 1:
                                do_ag(ag2_in0, ag2_out0)

            # ---- phase 3: edges layer 1 (+ fused dense layer 2) ----
            edge_phase((ag1_out0, ag1_out1), HID, root1_t, w2i_t, w2r_t, b2_t,
                       is_l2=False)

            # ---- phase 4: allgather h2' (second half) ----
            do_ag(ag2_in1, ag2_out1)

            # ---- phase 5: edges layer 2 ----
            edge_phase((ag2_out0, ag2_out1), NCLS, root2_t, None, None, None,
                       is_l2=True)

    nc.compile()
    _PROG_CACHE[meta] = nc
    return nc


# --------------------------------- kernel -----------------------------------

def kernel(**inputs):
    global LAST_EXEC_NS
    x = np.asarray(inputs["x"], np.float32)
    w1i = np.asarray(inputs["W1_init"], np.float32)
    w1r = np.asarray(inputs["W1_root"], np.float32)
    b1 = np.asarray(inputs["b1"], np.float32)
    w2i = np.asarray(inputs["W2_init"], np.float32)
    w2r = np.asarray(inputs["W2_root"], np.float32)
    b2 = np.asarray(inputs["b2"], np.float32)
    ei = np.asarray(inputs["edge_index"])

    in_maps, meta = _prep(x, w1i, w1r, b1, w2i, w2r, b2, ei)
    nc = _build(meta)

    trace = bool(int(os.environ.get("BASS_TRACE_KERNEL", "0")))
    r = run_bass_kernel_spmd(nc, in_maps, core_ids=list(range(NCORES)), trace=trace)
    if trace:
        LAST_EXEC_NS = r.exec_time_ns

    out = np.concatenate([r.results[c]["out"] for c in range(NCORES)], axis=0)
    return np.ascontiguousarray(out[:N]).astype(np.float32)


# revision 9
# speedup vs baseline: 1.7751x; 1.0430x over previous
"""Trainium2 Bass kernel for 2-layer ARMA GCN (nn_Net_33586644255234).

Strategy (graph/data parallel over 8 NeuronCores):
  - Nodes padded 40000 -> 40960 and sharded 5120/core (40 tiles of 128).
  - Weights replicated; per-core x^T shard shipped pre-transposed bf16.
  - Per layer:
      h' = dinv * (x @ W_init)          (dense, PE; dinv scale fused on ACT)
      AllGather h' -> full node table in every core's HBM
      per dst-tile: gather edge messages h'[src] with gpsimd.dma_gather,
      scatter-add via one-hot matmul:  psum += S_chunk^T @ msgs_chunk
      out = relu(dinv * psum + x @ W_root + b)
  - Edge bookkeeping (sort by dst tile, split by src half for int16 gather
    indices, chunk grids uniform across cores) is host-side sharding prep.

kernel(**inputs) takes FULL inputs, returns FULL [40000, 64] float32.
"""

import os
import sys

sys.path.insert(0, "/opt/trn_rl_repo")

import numpy as np
import ml_dtypes

import concourse.bass as bass
import concourse.mybir as mybir
import concourse.tile as tile
from concourse import bacc
from concourse.bass_utils import run_bass_kernel_spmd

# ---------------- problem constants (hardcoded per contract) ----------------
N, E, F_IN, HID, NCLS = 40000, 640000, 512, 128, 64
P = 128
NCORES = 8
NPC = 5120          # nodes per core (padded)
NPAD = NCORES * NPC  # 40960
TPC = NPC // P       # 40 dst tiles per core
KT = F_IN // P       # 4 k-tiles for layer-1 dense
HALF = NPAD // 2     # 20480 (int16-safe gather table half)
SB = 4               # dst tiles per gather superbatch

BF16 = mybir.dt.bfloat16
F32 = mybir.dt.float32
I16 = mybir.dt.int16

LAST_EXEC_NS = None  # set when BASS_TRACE=1


# ---------------------------- host preprocessing ----------------------------

def _wrap_idx(flat):
    """int16 flat index list -> dma_gather wrapped layout [128, len/16]."""
    L = flat.shape[0]
    assert L % 16 == 0
    w = flat.reshape(L // 16, 16).T  # [16, W]
    return np.tile(w, (8, 1)).copy()  # [128, W]


def _prep(x, w1i, w1r, b1, w2i, w2r, b2, edge_index):
    src = np.asarray(edge_index[0], np.int64)
    dst = np.asarray(edge_index[1], np.int64)

    deg = np.bincount(dst, minlength=N).astype(np.float32)
    dinv = np.where(deg > 0, 1.0 / np.sqrt(np.maximum(deg, 1.0)), 0.0).astype(
        np.float32
    )
    dinv_pad = np.zeros(NPAD, np.float32)
    dinv_pad[:N] = dinv

    tile_g = dst // P          # global dst tile 0..319
    s_rank = src // NPC
    s_q = src % NPC
    half = (s_q >= NPC // 2).astype(np.int64)
    s_local = (s_rank * (NPC // 2) + (s_q % (NPC // 2))).astype(np.int64)

    cnt = np.zeros((NCORES * TPC, 2), np.int64)
    np.add.at(cnt, (tile_g, half), 1)
    cnt3 = cnt.reshape(NCORES, TPC, 2)
    CH = np.ceil(cnt3.max(axis=0) / P).astype(np.int64)  # [TPC, 2] uniform
    CHA, CHB = CH[:, 0], CH[:, 1]
    CT = CHA + CHB
    colbase = np.concatenate([[0], np.cumsum(CT)])       # [TPC+1]
    TOTCH = int(colbase[-1])
    offA = np.concatenate([[0], np.cumsum(CHA)])         # chunks
    offB = np.concatenate([[0], np.cumsum(CHB)])
    LA, LB = int(offA[-1]) * P, int(offB[-1]) * P

    grp = tile_g * 2 + half
    order = np.argsort(grp, kind="stable")
    gs = grp[order]
    ss = src[order]
    ds = dst[order]
    gcnt = np.bincount(grp, minlength=NCORES * TPC * 2)
    gstart = np.concatenate([[0], np.cumsum(gcnt)])[:-1]
    pos = np.arange(E, dtype=np.int64) - gstart[gs]
    u = pos // P
    e = pos % P
    tg = gs // 2
    h = gs & 1
    core = tg // TPC
    tp = tg % TPC

    sl = s_local[order].astype(np.int16)
    gA = np.zeros((NCORES, max(LA, 16)), np.int16)
    gB = np.zeros((NCORES, max(LB, 16)), np.int16)
    mA = h == 0
    mB = ~mA
    flatA = (offA[tp[mA]] + u[mA]) * P + e[mA]
    flatB = (offB[tp[mB]] + u[mB]) * P + e[mB]
    gA[core[mA], flatA] = sl[mA]
    gB[core[mB], flatB] = sl[mB]

    dloc = (ds - tg * P).astype(np.float32)
    col = colbase[tp] + u + h * CHA[tp]
    dstl = np.full((NCORES, P, TOTCH), -1.0, np.float32)
    dstl[core, e, col] = dloc
    dstl = dstl.astype(ml_dtypes.bfloat16)

    # constants
    maxct = int(CT.max())
    iota = np.tile(np.arange(P, dtype=np.float32), maxct)
    iota = np.tile(iota[None, :], (P, 1)).astype(ml_dtypes.bfloat16)  # [128, maxct*128]
    ident = np.eye(P, dtype=ml_dtypes.bfloat16)

    # per-core tensors
    xpad = np.zeros((NPAD, F_IN), np.float32)
    xpad[:N] = x
    xT = np.ascontiguousarray(xpad.T)  # [512, 40960]

    in_maps = []
    for c in range(NCORES):
        xT_c = xT[:, c * NPC : (c + 1) * NPC].astype(ml_dtypes.bfloat16)
        dinv_c = dinv_pad[c * NPC : (c + 1) * NPC].reshape(TPC, P).T.copy()  # [128,40]
        in_maps.append(
            {
                "xT": np.ascontiguousarray(xT_c),
                "W1i": w1i.astype(ml_dtypes.bfloat16),
                "W1r": w1r.astype(ml_dtypes.bfloat16),
                "W2i": w2i.astype(ml_dtypes.bfloat16),
                "W2r": w2r.astype(ml_dtypes.bfloat16),
                "b1": b1.reshape(1, HID).astype(ml_dtypes.bfloat16),
                "b2": b2.reshape(1, NCLS).astype(ml_dtypes.bfloat16),
                "ones1": np.ones((1, P), ml_dtypes.bfloat16),
                "ident": ident,
                "iota": iota,
                "dinv": dinv_c,
                "dstl": np.ascontiguousarray(dstl[c]),
                "gidxA": _wrap_idx(gA[c]),
                "gidxB": _wrap_idx(gB[c]),
            }
        )

    meta = (tuple(int(v) for v in CHA), tuple(int(v) for v in CHB))
    return in_maps, meta


# ------------------------------ program build -------------------------------

_PROG_CACHE = {}


def _build(meta):
    if meta in _PROG_CACHE:
        return _PROG_CACHE[meta]

    CHA = np.array(meta[0])
    CHB = np.array(meta[1])
    CT = CHA + CHB
    colbase = np.concatenate([[0], np.cumsum(CT)])
    offA = np.concatenate([[0], np.cumsum(CHA)])
    offB = np.concatenate([[0], np.cumsum(CHB)])
    TOTCH = int(colbase[-1])
    maxct = int(CT.max())
    LA, LB = int(offA[-1]) * P, int(offB[-1]) * P
    nbatch = (TPC + SB - 1) // SB
    batches = [list(range(b * SB, min((b + 1) * SB, TPC))) for b in range(nbatch)]
    maxchA = max(int(CHA[b].sum()) for b in batches)
    maxchB = max(int(CHB[b].sum()) for b in batches)

    nc = bacc.Bacc("TRN2", target_bir_lowering=False, debug=False, num_devices=NCORES, num_swdge_queues=4)

    xT = nc.dram_tensor("xT", [F_IN, NPC], BF16, kind="ExternalInput")
    W1i = nc.dram_tensor("W1i", [F_IN, HID], BF16, kind="ExternalInput")
    W1r = nc.dram_tensor("W1r", [F_IN, HID], BF16, kind="ExternalInput")
    W2i = nc.dram_tensor("W2i", [HID, NCLS], BF16, kind="ExternalInput")
    W2r = nc.dram_tensor("W2r", [HID, NCLS], BF16, kind="ExternalInput")
    b1 = nc.dram_tensor("b1", [1, HID], BF16, kind="ExternalInput")
    b2 = nc.dram_tensor("b2", [1, NCLS], BF16, kind="ExternalInput")
    ones1 = nc.dram_tensor("ones1", [1, P], BF16, kind="ExternalInput")
    ident = nc.dram_tensor("ident", [P, P], BF16, kind="ExternalInput")
    iota = nc.dram_tensor("iota", [P, maxct * P], BF16, kind="ExternalInput")
    dinv = nc.dram_tensor("dinv", [P, TPC], F32, kind="ExternalInput")
    dstl = nc.dram_tensor("dstl", [P, TOTCH], BF16, kind="ExternalInput")
    gidxA = nc.dram_tensor("gidxA", [P, max(LA, 16) // 16], I16, kind="ExternalInput")
    gidxB = nc.dram_tensor("gidxB", [P, max(LB, 16) // 16], I16, kind="ExternalInput")
    out_d = nc.dram_tensor("out", [NPC, NCLS], F32, kind="ExternalOutput")

    RELU = mybir.ActivationFunctionType.Relu
    COPY = mybir.ActivationFunctionType.Copy
    EQ = mybir.AluOpType.is_equal
    ADD = mybir.AluOpType.add

    with tile.TileContext(nc) as tc:
        with (
            tc.tile_pool(name="const", bufs=1) as constp,
            tc.tile_pool(name="sbuf", bufs=3) as sbuf,
            tc.tile_pool(name="big", bufs=1) as bigp,
            tc.tile_pool(name="msgs", bufs=4) as msgp,
            tc.tile_pool(name="psumd", bufs=4, space="PSUM") as psumd,
            tc.tile_pool(name="psuma", bufs=2, space="PSUM") as psuma,
            tc.tile_pool(name="psumt", bufs=2, space="PSUM") as psumt,
            tc.tile_pool(name="dram", bufs=1, space="DRAM") as dram,
        ):
            # ---- constants to SBUF ----
            w1i_t = constp.tile([P, KT, HID], BF16)
            w1r_t = constp.tile([P, KT, HID], BF16)
            for k in range(KT):
                nc.sync.dma_start(out=w1i_t[:, k, :], in_=W1i[k * P : (k + 1) * P, :])
                nc.sync.dma_start(out=w1r_t[:, k, :], in_=W1r[k * P : (k + 1) * P, :])
            w2i_t = constp.tile([P, NCLS], BF16)
            w2r_t = constp.tile([P, NCLS], BF16)
            nc.sync.dma_start(out=w2i_t[:], in_=W2i[:, :])
            nc.sync.dma_start(out=w2r_t[:], in_=W2r[:, :])
            b1_t = constp.tile([1, HID], BF16)
            b2_t = constp.tile([1, NCLS], BF16)
            ones_t = constp.tile([1, P], BF16)
            nc.sync.dma_start(out=b1_t[:], in_=b1[:, :])
            nc.sync.dma_start(out=b2_t[:], in_=b2[:, :])
            nc.sync.dma_start(out=ones_t[:], in_=ones1[:, :])
            ident_t = constp.tile([P, P], BF16)
            nc.sync.dma_start(out=ident_t[:], in_=ident[:, :])
            iota_t = constp.tile([P, maxct * P], BF16)
            nc.sync.dma_start(out=iota_t[:], in_=iota[:, :])
            dinv_t = constp.tile([P, TPC], F32)
            nc.sync.dma_start(out=dinv_t[:], in_=dinv[:, :])
            dstl_t = constp.tile([P, TOTCH], BF16)
            nc.sync.dma_start(out=dstl_t[:], in_=dstl[:, :])
            gA_t = constp.tile([P, max(LA, 16) // 16], I16)
            gB_t = constp.tile([P, max(LB, 16) // 16], I16)
            nc.sync.dma_start(out=gA_t[:], in_=gidxA[:, :])
            nc.sync.dma_start(out=gB_t[:], in_=gidxB[:, :])

            # persistent per-layer SBUF
            root1_t = bigp.tile([P, TPC, HID], BF16)
            out1T_t = bigp.tile([P, TPC, HID], BF16)
            root2_t = bigp.tile([P, TPC, NCLS], BF16)

            HN = NPC // 2
            ag1_in0 = dram.tile([HN, HID], BF16)
            ag1_in1 = dram.tile([HN, HID], BF16)
            ag1_out0 = dram.tile([NPAD // 2, HID], BF16, addr_space="Shared")
            ag1_out1 = dram.tile([NPAD // 2, HID], BF16, addr_space="Shared")
            ag2_in0 = dram.tile([HN, P], BF16)
            ag2_in1 = dram.tile([HN, P], BF16)
            ag2_out0 = dram.tile([NPAD // 2, P], BF16, addr_space="Shared")
            ag2_out1 = dram.tile([NPAD // 2, P], BF16, addr_space="Shared")

            pending_ags = []

            def do_ag(src, dst):
                inst = nc.gpsimd.collective_compute(
                    "AllGather",
                    mybir.AluOpType.bypass,
                    replica_groups=[list(range(NCORES))],
                    ins=[src.opt()],
                    outs=[dst.opt()],
                )
                pending_ags.append(inst)
                return inst

            def order_after_ags(inst):
                while pending_ags:
                    tile.add_dep_helper(
                        inst.ins, pending_ags.pop().ins, sync=False,
                        reason="gather after AG trigger",
                    )

            # ---- phase 1: dense layer 1 ----
            for b in range(TPC):
                xb = sbuf.tile([P, KT, P], BF16, tag="xb")
                for k in range(KT):
                    nc.sync.dma_start(
                        out=xb[:, k, :],
                        in_=xT[k * P : (k + 1) * P, b * P : (b + 1) * P],
                    )
                ps_i = psumd.tile([P, HID], F32, tag="d")
                ps_r = psumd.tile([P, HID], F32, tag="d")
                for k in range(KT):
                    nc.tensor.matmul(
                        ps_i[:], lhsT=xb[:, k, :], rhs=w1i_t[:, k, :],
                        start=(k == 0), stop=(k == KT - 1),
                    )
                for k in range(KT):
                    nc.tensor.matmul(
                        ps_r[:], lhsT=xb[:, k, :], rhs=w1r_t[:, k, :],
                        start=(k == 0), stop=False,
                    )
                nc.tensor.matmul(ps_r[:], lhsT=ones_t[:], rhs=b1_t[:],
                                 start=False, stop=True)
                hb = sbuf.tile([P, HID], BF16, tag="hb")
                nc.scalar.activation(hb[:], ps_i[:], COPY,
                                     scale=dinv_t[:, b : b + 1])
                nc.scalar.activation(root1_t[:, b, :], ps_r[:], COPY)
                if b < TPC // 2:
                    nc.sync.dma_start(out=ag1_in0[b * P : (b + 1) * P, :], in_=hb[:])
                else:
                    bb = b - TPC // 2
                    nc.sync.dma_start(out=ag1_in1[bb * P : (bb + 1) * P, :], in_=hb[:])
                if b == TPC // 2 - 1:
                    do_ag(ag1_in0, ag1_out0)
            # ---- phase 2: allgather h' (second half) ----
            do_ag(ag1_in1, ag1_out1)

            # ---- edge phase helper ----
            def edge_phase(tables, fdim, root_t, w_i, w_r, bias_t, is_l2):
                for bi, bt in enumerate(batches):
                    nA = int(CHA[bt].sum())
                    nB = int(CHB[bt].sum())
                    qA = (2 * bi) % 4
                    qB = (2 * bi + 1) % 4
                    mA_t = msgp.tile([P, max(maxchA, 1), P], BF16, tag="mA")
                    mB_t = msgp.tile([P, max(maxchB, 1), P], BF16, tag="mB")
                    if nA:
                        a0 = int(offA[bt[0]])
                        gi = nc.gpsimd.dma_gather(
                            out_ap=mA_t[:, :nA, :],
                            in_ap=tables[0][:, :],
                            idxs_ap=gA_t[:, a0 * 8 : (a0 + nA) * 8],
                            num_idxs=nA * P,
                            num_idxs_reg=nA * P,
                            elem_size=P,
                            single_packet=False,
                            queue_num=qA,
                        )
                        order_after_ags(gi)
                    if nB:
                        b0 = int(offB[bt[0]])
                        gi = nc.gpsimd.dma_gather(
                            out_ap=mB_t[:, :nB, :],
                            in_ap=tables[1][:, :],
                            idxs_ap=gB_t[:, b0 * 8 : (b0 + nB) * 8],
                            num_idxs=nB * P,
                            num_idxs_reg=nB * P,
                            elem_size=P,
                            single_packet=False,
                            queue_num=qB,
                        )
                        order_after_ags(gi)
                    aoff = boff = 0
                    for t in bt:
                        ct = int(CT[t])
                        ca, cb = int(CHA[t]), int(CHB[t])
                        if ct == 0:
                            continue
                        s_t = sbuf.tile([P, maxct, P], BF16, tag="s")
                        nc.vector.tensor_tensor(
                            out=s_t[:, :ct, :],
                            in0=iota_t[:, : ct * P],
                            in1=dstl_t[:, colbase[t] : colbase[t] + ct].to_broadcast(
                                [P, ct, P]
                            ),
                            op=EQ,
                        )
                        ps_a = psuma.tile([P, fdim], F32, tag="a")
                        for u in range(ct):
                            if u < ca:
                                rhs = mA_t[:, aoff + u, :fdim]
                            else:
                                rhs = mB_t[:, boff + (u - ca), :fdim]
                            nc.tensor.matmul(
                                ps_a[:], lhsT=s_t[:, u, :], rhs=rhs,
                                start=(u == 0), stop=(u == ct - 1),
                            )
                        aoff += ca
                        boff += cb
                        # epilogue: relu(dinv*agg + root)
                        tt = sbuf.tile([P, fdim], BF16, tag="tt")
                        nc.scalar.activation(tt[:], ps_a[:], COPY,
                                             scale=dinv_t[:, t : t + 1])
                        o_pre = sbuf.tile([P, fdim], BF16, tag="opre")
                        nc.vector.tensor_tensor(
                            out=o_pre[:], in0=tt[:], in1=root_t[:, t, :], op=ADD
                        )
                        o_t = sbuf.tile([P, fdim], F32 if is_l2 else BF16, tag="o")
                        nc.scalar.activation(o_t[:], o_pre[:], RELU)
                        if is_l2:
                            nc.sync.dma_start(
                                out=out_d[t * P : (t + 1) * P, :], in_=o_t[:]
                            )
                        else:
                            # transpose for layer-2 dense
                            ps_t = psumt.tile([P, P], BF16, tag="t")
                            nc.tensor.transpose(ps_t[:], o_t[:], ident_t[:])
                            nc.scalar.activation(out1T_t[:, t, :], ps_t[:], COPY)
                            # layer-2 dense for this tile
                            ps_h2 = psumd.tile([P, NCLS], F32, tag="d")
                            ps_r2 = psumd.tile([P, NCLS], F32, tag="d")
                            nc.tensor.matmul(
                                ps_h2[:], lhsT=out1T_t[:, t, :], rhs=w_i[:],
                                start=True, stop=True,
                            )
                            nc.tensor.matmul(
                                ps_r2[:], lhsT=out1T_t[:, t, :], rhs=w_r[:],
                                start=True, stop=False,
                            )
                            nc.tensor.matmul(
                                ps_r2[:], lhsT=ones_t[:], rhs=bias_t[:],
                                start=False, stop=True,
                            )
                            h2b = sbuf.tile([P, NCLS], BF16, tag="h2b")
                            nc.scalar.activation(h2b[:], ps_h2[:], COPY,
                                                 scale=dinv_t[:, t : t + 1])
                            nc.scalar.activation(root2_t[:, t, :], ps_r2[:], COPY)
                            if t < TPC // 2:
                                nc.sync.dma_start(
                                    out=ag2_in0[t * P : (t + 1) * P, :NCLS],
                                    in_=h2b[:],
                                )
                            else:
                                t2 = t - TPC // 2
                                nc.sync.dma_start(
                                    out=ag2_in1[t2 * P : (t2 + 1) * P, :NCLS],
                                    in_=h2b[:],
                                )
                            if t == TPC // 2 - 1:
                                do_ag(ag2_in0, ag2_out0)

            # ---- phase 3: edges layer 1 (+ fused dense layer 2) ----
            edge_phase((ag1_out0, ag1_out1), HID, root1_t, w2i_t, w2r_t, b2_t,
                       is_l2=False)

            # ---- phase 4: allgather h2' (second half) ----
            do_ag(ag2_in1, ag2_out1)

            # ---- phase 5: edges layer 2 ----
            edge_phase((ag2_out0, ag2_out1), NCLS, root2_t, None, None, None,
                       is_l2=True)

    nc.compile()
    _PROG_CACHE[meta] = nc
    return nc


# --------------------------------- kernel -----------------------------------

def kernel(**inputs):
    global LAST_EXEC_NS
    x = np.asarray(inputs["x"], np.float32)
    w1i = np.asarray(inputs["W1_init"], np.float32)
    w1r = np.asarray(inputs["W1_root"], np.float32)
    b1 = np.asarray(inputs["b1"], np.float32)
    w2i = np.asarray(inputs["W2_init"], np.float32)
    w2r = np.asarray(inputs["W2_root"], np.float32)
    b2 = np.asarray(inputs["b2"], np.float32)
    ei = np.asarray(inputs["edge_index"])

    in_maps, meta = _prep(x, w1i, w1r, b1, w2i, w2r, b2, ei)
    nc = _build(meta)

    trace = bool(int(os.environ.get("BASS_TRACE_KERNEL", "0")))
    r = run_bass_kernel_spmd(nc, in_maps, core_ids=list(range(NCORES)), trace=trace)
    if trace:
        LAST_EXEC_NS = r.exec_time_ns

    out = np.concatenate([r.results[c]["out"] for c in range(NCORES)], axis=0)
    return np.ascontiguousarray(out[:N]).astype(np.float32)
